# revision 1
# baseline (speedup 1.0000x reference)
"""GCN 2-layer + linear head on 8 Trainium2 NeuronCores (Bass/Tile).

v2: multi-pass scatter-add + ReduceScatter architecture.

- Edges partitioned by SOURCE shard; normalization folded node-wise
  (hs = dinv * (x@W)), self-loop handled as a regular edge:
  out[d] = b + dinv[d] * sum_{e: dst=d, incl. self} hs[src_e]
- Each core scatter-adds its edges' hs rows into a GLOBAL pair-packed partial
  table PT[50176, 128] f16 (node i of shard s -> pair-row 6272*s + (i%12500)//2,
  col half (i%2)*64). One fp16 ReduceScatter per layer hands core k the summed
  rows of its own shard (chunk k = pair-rows [6272k, 6272(k+1))).
- The scatter-add needs NO per-edge gather and NO message materialization:
  its sequential SBUF source is the hs table itself.  Edges are split into 4
  classes m = (dst-group g in {0,1}) x (dst-parity c); within a class each
  node's edges get ranks j.  Pass (m,j) is one dma_scatter_add over an exact
  PREFIX of the class-m hs table (nodes sorted by class-m degree desc);
  int16 idx = pair-row within the 25088-row group window; middle inactive
  positions point at a spare dump row.
- Layer 2 reuses the identical pass structure; PT is never re-zeroed -- the
  final epilogue uses RS2 - RS1.

SPMD: all 8 cores run one program; per-pass prefix sizes are maxed over cores.
"""

import numpy as np

import concourse.bacc as bacc
import concourse.mybir as mybir
import concourse.tile as tile
from concourse.bass_utils import run_bass_kernel_spmd

# problem shapes (hardcoded per contract)
N = 100000
E = 1600000
FIN = 128
HID = 64

NC_ = 8
P = 128
NLOC = N // NC_                 # 12500
NPOS = 12544                    # padded positions (98 * 128)
NROW = NPOS // P                # 98
PAIRS_SHARD = 6272              # pair-rows per shard (6250 real + spares)
DUMP_PAIR = 6250                # spare pair-row, relative to shard 4g base
PT_ROWS = NC_ * PAIRS_SHARD     # 50176
GW = 4 * PAIRS_SHARD            # 25088 pair-rows per scatter group
RS_OUT = PT_ROWS // NC_         # 6272 pair-rows per core
TPC = RS_OUT // P               # 49 pair-tiles per core

# knockout flags for cost attribution (profiling only; breaks correctness)
SKIP_SCATTER = False
SCATTER_LIMIT = 10 ** 9     # emit only the first N calls per layer (debug)
SKIP_RS = False
SKIP_PH1 = False
SKIP_G2 = False


# ----------------------------------------------------------------- host prep
def _wrap16(a):
    """int idx array (len % 16 == 0) -> SWDGE wrapped [128, n/16] int16."""
    a = np.asarray(a, np.int16)
    w = a.reshape(-1, 16).T
    return np.tile(w, (8, 1))


def _prep(x, edge_index):
    x = np.asarray(x, np.float32)
    src_g = np.asarray(edge_index[0], np.int64)
    dst_g = np.asarray(edge_index[1], np.int64)

    deg = np.bincount(dst_g, minlength=N).astype(np.float32) + 1.0
    dinv = (1.0 / np.sqrt(deg)).astype(np.float32)

    per_core = []
    for k in range(NC_):
        m_ = (src_g // NLOC) == k
        s = np.concatenate([src_g[m_] % NLOC, np.arange(NLOC)])
        d = np.concatenate([dst_g[m_], np.arange(NLOC) + k * NLOC])
        ds = d // NLOC
        di = d % NLOC
        cls = (ds // 4) * 2 + (di % 2)        # class m in 0..3
        pairrow = 6272 * (ds % 4) + di // 2   # group-relative pair row
        degm = np.zeros((4, NLOC), np.int64)
        np.add.at(degm, (cls, s), 1)
        per_core.append((s, cls, pairrow, degm))

    # Per (core, class): give each source's class-m edges distinct ranks and
    # make every rank's destination set unique (HW scatter-add RMW is not
    # atomic across duplicate destinations within one call): 2-swaps within
    # the source's own rank set, then move losers to the lowest free
    # (rank, dst) slot.  sigma then sorts sources by max used rank (desc) so
    # each pass's active set is an exact position prefix; rank holes become
    # cheap dump descriptors.
    core_rounds = [[None] * 4 for _ in range(NC_)]
    core_meta = []
    for k in range(NC_):
        s, cls, pairrow, degm = per_core[k]
        sigmas, poss = [], []
        for m in range(4):
            sel = cls == m
            es, epr = s[sel], pairrow[sel]
            order = np.lexsort((epr, es))
            sm, ep = es[order], epr[order]
            ne = len(sm)
            newn = np.r_[True, sm[1:] != sm[:-1]]
            run = np.maximum.accumulate(np.where(newn, np.arange(ne), 0))
            rank = (np.arange(ne) - run).astype(np.int64)

            KEY = 1 << 20
            key = rank * KEY + ep
            o = np.argsort(key, kind="stable")
            ks = key[o]
            dup = np.zeros(ne, bool)
            dup[o] = np.r_[False, ks[1:] == ks[:-1]]
            occ = {}                       # (rank*KEY+dst) -> owning edge
            for i in np.nonzero(~dup)[0]:
                occ[int(key[i])] = int(i)
            idx_by_src = {}
            for i in range(ne):
                idx_by_src.setdefault(int(sm[i]), []).append(i)
            pending = [int(i) for i in np.nonzero(dup)[0]]
            for sweep in range(6):
                if not pending:
                    break
                nxt = []
                for i in pending:
                    ki = int(rank[i]) * KEY + int(ep[i])
                    if occ.get(ki) == i:
                        continue           # became valid via a swap partner
                    done = False
                    for jj in idx_by_src[int(sm[i])]:
                        if jj == i or rank[jj] == rank[i]:
                            continue
                        kjj = int(rank[jj]) * KEY + int(ep[jj])
                        if occ.get(kjj) != jj:
                            continue
                        k1 = int(rank[jj]) * KEY + int(ep[i])
                        k2 = int(rank[i]) * KEY + int(ep[jj])
                        if k1 not in occ and k2 not in occ:
                            del occ[kjj]
                            rank[i], rank[jj] = rank[jj], rank[i]
                            occ[k1] = i
                            occ[k2] = jj
                            done = True
                            break
                    if not done:
                        nxt.append(i)
                pending = nxt
            # tier 2: lowest free (rank, dst) slot for this source
            for i in pending:
                ki = int(rank[i]) * KEY + int(ep[i])
                if occ.get(ki) == i:
                    continue
                used = {int(rank[jj]) for jj in idx_by_src[int(sm[i])]}
                r = 0
                while r in used or (r * KEY + int(ep[i])) in occ:
                    r += 1
                rank[i] = r
                occ[r * KEY + int(ep[i])] = i

            # sigma: sort sources by max used rank (desc) -> exact prefixes
            maxrank = np.full(NLOC, -1, np.int64)
            np.maximum.at(maxrank, sm, rank)
            sig = np.argsort(-maxrank, kind="stable")
            pos = np.empty(NLOC, np.int64)
            pos[sig] = np.arange(NLOC)
            sigmas.append(sig)
            poss.append(pos)
            pm = pos[sm]

            rounds = []
            for j in range(int(rank.max()) + 1):
                ss = rank == j
                pj, ej = pm[ss], ep[ss]
                assert np.unique(ej).size == len(ej), "dst collision left"
                assert np.unique(pj).size == len(pj), "src collision left"
                rounds.append((pj, ej))
            core_rounds[k][m] = rounds
        core_meta.append((sigmas, poss))

    J = [max(len(core_rounds[k][m]) for k in range(NC_)) for m in range(4)]
    n_mj = [np.zeros(J[m], np.int64) for m in range(4)]
    for k in range(NC_):
        for m in range(4):
            for j, (p2, _e2) in enumerate(core_rounds[k][m]):
                if len(p2):
                    n_mj[m][j] = max(n_mj[m][j], int(p2.max()) + 1)

    # static call list interleaved across classes (round-robin by pass):
    # adjacent calls hit different groups so their DMAs can overlap.
    # Calls over SCAP descriptors hang the scatter-add ucode on HW -> split
    # into tile-aligned sub-calls (t0 = source tile offset).
    SCAP = 6272
    calls, off = [], 0
    for j in range(max(J)):
        chunks = {}
        for m in (0, 2, 1, 3):
            if j >= J[m]:
                continue
            n = int(n_mj[m][j])
            base = off
            off += (n + 15) // 16
            for ci, c0 in enumerate(range(0, n, SCAP)):
                ncnk = min(SCAP, n - c0)
                chunks.setdefault(ci, []).append(
                    (m, j, ncnk, (ncnk + P - 1) // P,
                     base + c0 // 16, c0 // P))
        for ci in sorted(chunks):
            calls.extend(chunks[ci])
    ti_cols = off

    in_maps = []
    for k in range(NC_):
        sigmas, poss = core_meta[k]
        idx_cols, seen = [], set()
        for (m, j, n, nt, _o, t0) in calls:
            if (m, j) in seen:
                continue               # sub-calls share the (m, j) idx array
            seen.add((m, j))
            nfull = int(n_mj[m][j])
            a = np.full(((nfull + 15) // 16) * 16, -1, np.int64)
            a[:nfull] = DUMP_PAIR
            if j < len(core_rounds[k][m]):
                p2, e2 = core_rounds[k][m][j]
                a[p2] = e2
            assert (a[:nfull] >= 0).all() and (a[:nfull] < GW).all()
            idx_cols.append(a)
        idx = np.concatenate([_wrap16(c) for c in idx_cols], axis=1)

        xTs, sgidx = [], []
        xk = x[k * NLOC:(k + 1) * NLOC]
        dk = dinv[k * NLOC:(k + 1) * NLOC]
        for m in range(4):
            sig = sigmas[m]
            xT = np.zeros((FIN, NPOS), np.float16)
            xT[:, :NLOC] = (xk[sig] * dk[sig][:, None]).T
            xTs.append(xT)
            gi = np.full(NPOS, NLOC, np.int64)
            gi[:NLOC] = sig
            sgidx.append(_wrap16(gi))

        # dinv in pair layout [128, 49, 128]: node n = 98p + 2t + (col//64)
        nn = (98 * np.arange(P)[:, None, None]
              + 2 * np.arange(TPC)[None, :, None]
              + (np.arange(P)[None, None, :] // 64))
        dpad = np.zeros(NPOS + 1, np.float32)
        dpad[:NLOC] = dk
        dinvpair = dpad[np.minimum(nn, NPOS)].astype(np.float16)

        in_maps.append({
            "xT0": xTs[0], "xT1": xTs[1], "xT2": xTs[2], "xT3": xTs[3],
            "sg0": sgidx[0], "sg1": sgidx[1], "sg2": sgidx[2], "sg3": sgidx[3],
            "idx": idx,
            "dinvpair": dinvpair.reshape(P, TPC * P),
        })
    return in_maps, calls, ti_cols


# ------------------------------------------------------------- device build
def _build(calls, ti_cols):
    f32, f16, i16 = mybir.dt.float32, mybir.dt.float16, mybir.dt.int16
    nc = bacc.Bacc("TRN2", num_devices=NC_)

    xTs = [nc.dram_tensor(f"xT{m}", [FIN, NPOS], f16, kind="ExternalInput")
           for m in range(4)]
    sgs = [nc.dram_tensor(f"sg{m}", [P, NPOS // 16], i16, kind="ExternalInput")
           for m in range(4)]
    idx = nc.dram_tensor("idx", [P, ti_cols], i16, kind="ExternalInput")
    dinvpair = nc.dram_tensor("dinvpair", [P, TPC * P], f16,
                              kind="ExternalInput")
    W1 = nc.dram_tensor("W1", [FIN, HID], f32, kind="ExternalInput")
    W2 = nc.dram_tensor("W2", [HID, HID], f32, kind="ExternalInput")
    b1rep = nc.dram_tensor("b1rep", [P, P], f16, kind="ExternalInput")
    b2rep = nc.dram_tensor("b2rep", [P, P], f16, kind="ExternalInput")
    Wcrep = nc.dram_tensor("Wcrep", [P, P], f16, kind="ExternalInput")
    bcrep = nc.dram_tensor("bcrep", [P, 1], f32, kind="ExternalInput")
    out = nc.dram_tensor("out", [P, NROW], f32, kind="ExternalOutput")

    copy_ = mybir.ActivationFunctionType.Copy
    mult = mybir.AluOpType.mult
    add = mybir.AluOpType.add

    with tile.TileContext(nc) as tc:
        with (
            tc.tile_pool(name="cst", bufs=1) as cst,
            tc.tile_pool(name="io", bufs=1) as io,
            tc.tile_pool(name="h1p", bufs=1) as h1p,
            tc.tile_pool(name="work", bufs=1) as work,
            tc.tile_pool(name="ph", bufs=4, space="PSUM") as php,
            tc.tile_pool(name="dram", bufs=1, space="DRAM") as dram,
        ):
            # ---------------- constants
            W1sb = cst.tile([FIN, HID], f32)
            nc.sync.dma_start(W1sb[:], W1[:])
            W2sb = cst.tile([HID, HID], f32)
            nc.sync.dma_start(W2sb[:], W2[:])
            W2h = cst.tile([HID, HID], f16)
            nc.vector.tensor_copy(out=W2h[:], in_=W2sb[:])
            b1sb = cst.tile([P, P], f16)
            nc.sync.dma_start(b1sb[:], b1rep[:])
            b2sb = cst.tile([P, P], f16)
            nc.sync.dma_start(b2sb[:], b2rep[:])
            Wcsb = cst.tile([P, P], f16)
            nc.sync.dma_start(Wcsb[:], Wcrep[:])
            bcsb = cst.tile([P, 1], f32)
            nc.sync.dma_start(bcsb[:], bcrep[:])
            W1h = cst.tile([FIN, HID], f16)
            nc.vector.tensor_copy(out=W1h[:], in_=W1sb[:])
            ixall = cst.tile([P, ti_cols], i16)
            nc.sync.dma_start(ixall[:], idx[:])
            dpsb = cst.tile([P, TPC * P], f16)
            nc.sync.dma_start(dpsb[:], dinvpair[:])
            sgsb = []
            for m in range(4):
                t_ = cst.tile([P, NPOS // 16], i16, name=f"sg{m}sb")
                nc.sync.dma_start(t_[:], sgs[m][:])
                sgsb.append(t_)

            # ---------------- DRAM scratch
            PT = dram.tile([PT_ROWS, P], f16)
            rs1 = dram.tile([RS_OUT, P], f16)
            rs2 = dram.tile([RS_OUT, P], f16)
            h1nat = dram.tile([NPOS + 16, P], f16)

            # zero PT (incl. spares): per-group chunks from a zeroed work
            # tile (group 0 rows first so its scatters can start earlier)
            zsb = work.tile([P, TPC, P], f16, name="wA")
            nc.vector.memset(zsb[:], 0.0)
            zflat = zsb[:].rearrange("p t c -> p (t c)")
            for g in range(2):
                PTz = PT[g * GW:(g + 1) * GW, :].rearrange(
                    "(p a) c -> p (a c)", p=P)
                for i in range(4):
                    nc.scalar.dma_start(
                        PTz[:, i * 6272:(i + 1) * 6272], zflat)

            # ---------------- phase 1: hs1_m = (dinv*x)_m @ W1  (f16)
            # (the same 4 tiles are overwritten with hs2 in layer 2)
            hs1 = [cst.tile([P, NROW, HID], f16, name=f"hs_{m}")
                   for m in range(4)]
            BB = 7                      # r-tiles per PSUM eviction batch
            for m in range(4):
                xsb = io.tile([FIN, NPOS], f16, name="xsb")
                if not SKIP_PH1:
                    nc.sync.dma_start(xsb[:], xTs[m][:])
                for r0 in range(0, NROW, BB):
                    nb = min(BB, NROW - r0)
                    ph = php.tile([P, BB, HID], f32, name="ph")
                    for i in range(nb):
                        r = r0 + i
                        nc.tensor.matmul(out=ph[:, i, :],
                                         lhsT=xsb[:, r * P:(r + 1) * P],
                                         rhs=W1h[:], start=True, stop=True)
                    if (r0 // BB) % 2 == 0:
                        nc.vector.tensor_copy(out=hs1[m][:, r0:r0 + nb, :],
                                              in_=ph[:, :nb, :])
                    else:
                        nc.scalar.activation(out=hs1[m][:, r0:r0 + nb, :],
                                             in_=ph[:, :nb, :], func=copy_)

            hs2 = None
            for L in range(2):
                hsL = hs1 if L == 0 else hs2
                for ci, (m, j, n, nt, off, t0) in enumerate(calls):
                    if SKIP_SCATTER or ci >= SCATTER_LIMIT:
                        break
                    g, c = m // 2, m % 2
                    n16 = (n + 15) // 16
                    nc.gpsimd.dma_scatter_add(
                        PT[g * GW:(g + 1) * GW, c * HID:(c + 1) * HID],
                        hsL[m][:, t0:t0 + nt, :],
                        ixall[:, off:off + n16],
                        n, n, HID, elem_step=P,
                        single_packet=False)
                rs = rs1 if L == 0 else rs2
                if not SKIP_RS:
                    nc.gpsimd.collective_compute(
                        "ReduceScatter", add,
                        replica_groups=[list(range(NC_))],
                        ins=[PT[:]], outs=[rs[:]],
                    )

                if L == 0:
                    # ---- epilogue 1 (pair layout, natural order), 2 chunks
                    rsv = rs1[:].rearrange("(p t) c -> p t c", p=P)
                    dpv = dpsb[:].rearrange("p (t c) -> p t c", c=P)
                    h1v = h1nat[:NPOS, :HID].rearrange(
                        "(p n) f -> p n f", p=P)
                    for h_ in range(2):
                        tl, th = (0, 25) if h_ == 0 else (25, TPC)
                        r1 = work.tile([P, TPC, P], f16, name="wA")
                        nc.sync.dma_start(r1[:, tl:th, :], rsv[:, tl:th, :])
                        e1 = work.tile([P, TPC, P], f16, name="wB")
                        nc.vector.tensor_tensor(
                            out=e1[:, tl:th, :], in0=r1[:, tl:th, :],
                            in1=dpv[:, tl:th, :], op=mult)
                        e1b = work.tile([P, TPC, P], f16, name="wC")
                        nc.vector.tensor_tensor(
                            out=e1b[:, tl:th, :], in0=e1[:, tl:th, :],
                            in1=b1sb[:, None, :].to_broadcast([P, th - tl, P]),
                            op=add)
                        h1pr = work.tile([P, TPC, P], f16, name="wA")
                        nc.vector.tensor_scalar(
                            out=h1pr[:, tl:th, :], in0=e1b[:, tl:th, :],
                            scalar1=0.0, scalar2=None,
                            op0=mybir.AluOpType.max)
                        # pre-scale by dinv: dinv*(h1@W2) == (dinv*h1)@W2
                        h1sc = work.tile([P, TPC, P], f16, name="wB")
                        nc.vector.tensor_tensor(
                            out=h1sc[:, tl:th, :], in0=h1pr[:, tl:th, :],
                            in1=dpv[:, tl:th, :], op=mult)
                        nc.sync.dma_start(
                            h1v[:, 2 * tl:2 * th, :],
                            h1sc[:, tl:th, :].rearrange(
                                "p t (c f) -> p (t c) f", c=2))

                    # ---- phase 2 per class (reuse the hs tiles)
                    hs2 = hs1
                    for m in range(4):
                        h1T = h1p.tile([P, 1, NPOS], f16, name="h1T")
                        if not SKIP_G2:
                            nc.gpsimd.dma_gather(
                                h1T[:], h1nat[:], sgsb[m][:], NPOS, NPOS, P,
                                transpose=True, single_packet=False)
                        for r0 in range(0, NROW, BB):
                            nb = min(BB, NROW - r0)
                            ph2 = php.tile([P, BB, HID], f32, name="ph")
                            for i in range(nb):
                                r = r0 + i
                                nc.tensor.matmul(
                                    out=ph2[:, i, :],
                                    lhsT=h1T[:HID, 0, r * P:(r + 1) * P],
                                    rhs=W2h[:], start=True, stop=True)
                            if (r0 // BB) % 2 == 0:
                                nc.vector.tensor_copy(
                                    out=hs2[m][:, r0:r0 + nb, :],
                                    in_=ph2[:, :nb, :])
                            else:
                                nc.scalar.activation(
                                    out=hs2[m][:, r0:r0 + nb, :],
                                    in_=ph2[:, :nb, :], func=copy_)
                else:
                    # ---- final epilogue + classifier, 2 chunks
                    r1v = rs1[:].rearrange("(p t) c -> p t c", p=P)
                    r2v = rs2[:].rearrange("(p t) c -> p t c", p=P)
                    dpv2 = dpsb[:].rearrange("p (t c) -> p t c", c=P)
                    oc = work.tile([P, TPC * 2], f32, name="oc")
                    for h_ in range(2):
                        tl, th = (0, 25) if h_ == 0 else (25, TPC)
                        w_ = th - tl
                        r1b = work.tile([P, TPC, P], f16, name="wA")
                        nc.sync.dma_start(r1b[:, tl:th, :], r1v[:, tl:th, :])
                        r2 = work.tile([P, TPC, P], f16, name="wB")
                        nc.sync.dma_start(r2[:, tl:th, :], r2v[:, tl:th, :])
                        dd = work.tile([P, TPC, P], f16, name="wC")
                        nc.vector.tensor_tensor(
                            out=dd[:, tl:th, :], in0=r2[:, tl:th, :],
                            in1=r1b[:, tl:th, :],
                            op=mybir.AluOpType.subtract)
                        e2 = work.tile([P, TPC, P], f16, name="wA")
                        nc.vector.tensor_tensor(
                            out=e2[:, tl:th, :], in0=dd[:, tl:th, :],
                            in1=dpv2[:, tl:th, :], op=mult)
                        e2b = work.tile([P, TPC, P], f16, name="wB")
                        nc.vector.tensor_tensor(
                            out=e2b[:, tl:th, :], in0=e2[:, tl:th, :],
                            in1=b2sb[:, None, :].to_broadcast([P, w_, P]),
                            op=add)
                        h2pr = work.tile([P, TPC, P], f16, name="wC")
                        nc.vector.tensor_scalar(
                            out=h2pr[:, tl:th, :], in0=e2b[:, tl:th, :],
                            scalar1=0.0, scalar2=None,
                            op0=mybir.AluOpType.max)
                        hw_ = work.tile([P, TPC, P], f16, name="wA")
                        nc.vector.tensor_tensor(
                            out=hw_[:, tl:th, :], in0=h2pr[:, tl:th, :],
                            in1=Wcsb[:, None, :].to_broadcast([P, w_, P]),
                            op=mult)
                        nc.vector.tensor_reduce(
                            out=oc[:, 2 * tl:2 * th],
                            in_=hw_[:, tl:th, :].rearrange(
                                "p t (h f) -> p (t h) f", h=2),
                            axis=mybir.AxisListType.X, op=add)
                    ocb = work.tile([P, TPC * 2], f32, name="ocb")
                    nc.vector.tensor_scalar(
                        out=ocb[:], in0=oc[:], scalar1=bcsb[:, :1],
                        scalar2=None, op0=add)
                    nc.sync.dma_start(out[:], ocb[:])

    nc.compile()
    return nc


_CACHE = {}


def kernel(x, edge_index, W1, b1, W2, b2, Wc, bc):
    x = np.asarray(x, np.float32)
    edge_index = np.asarray(edge_index, np.int32)
    in_maps, calls, ti_cols = _prep(x, edge_index)

    key = (ti_cols, tuple(c[:4] + c[5:] for c in calls))
    if key not in _CACHE:
        _CACHE[key] = _build(calls, ti_cols)
    nc = _CACHE[key]

    b1f = np.asarray(b1, np.float32).reshape(HID)
    b2f = np.asarray(b2, np.float32).reshape(HID)
    wcf = np.asarray(Wc, np.float32).reshape(HID)
    shared = {
        "W1": np.asarray(W1, np.float32),
        "W2": np.asarray(W2, np.float32),
        "b1rep": np.tile(np.concatenate([b1f, b1f]), (P, 1)).astype(np.float16),
        "b2rep": np.tile(np.concatenate([b2f, b2f]), (P, 1)).astype(np.float16),
        "Wcrep": np.tile(np.concatenate([wcf, wcf]), (P, 1)).astype(np.float16),
        "bcrep": np.full((P, 1), np.asarray(bc, np.float32).reshape(()),
                         np.float32),
    }
    for m_ in in_maps:
        m_.update(shared)

    res = run_bass_kernel_spmd(nc, in_maps, core_ids=list(range(NC_)))
    # out[p, j] is node n = 98p + j of core k
    return np.concatenate(
        [res.results[k]["out"].reshape(-1)[:NLOC] for k in range(NC_)]
    ).astype(np.float32)

